# revision 11
# baseline (speedup 1.0000x reference)
"""2-layer GAT (8 heads x 32) on 8 Trainium2 NeuronCores.

Strategy (dst-major "round" tiling):
 - Host permutes nodes into 8 core-shards x 49 blocks x 128 slots, packing
   nodes with similar (lo,hi) in-degree into the same block rank so the
   per-rank round counts (ELL padding) are small and identical across cores
   (single SPMD NEFF).
 - Gather tables hold per-node rows [h_proj fp16 (256) | el fp32 (8) | pad]
   = 768B, gathered per edge with the TIE-accelerated dma_gather (int16
   indices -> table split at row 32768 into lo/hi halves; each node's
   lo-src edges occupy the first R_lo rounds of its block, hi-src edges the
   rest, padded with dummy rows whose el = -1e30 so exp() -> 0).
 - Round tile t of a block holds edge t of each of the 128 dst nodes =>
   attention is fully dst-major: er is a direct per-partition [128,8] tile,
   softmax max/sum are strided free-dim reductions, and the weighted
   aggregation is an identity-lhsT matmul accumulating G_t into PSUM.
 - Tables are built shard-local (x^T @ [W|wa_l|wa_r] fused matmul) and
   AllGather'd between layers.
"""
import sys
sys.path.insert(0, "/opt/trn_rl_repo")
import numpy as np

N = 50000
E_FULL = 850000
IN_F = 128
H = 8
D = 32
HID = 256
P = 128
NC = 8
NBLK = 49
CAP = NBLK * P          # 6272 slots per core
REAL = 6250             # real nodes per core
NTOT = NC * CAP         # 50176
LO = 32768
ROWE = 384              # fp16 elems per table row (768B)
LO_DUMMY = 6250         # core0 spare slot (block 48, slot 106)
HI_DUMMY = 7 * CAP + 6250   # 50154 -> hi-relative 17386
CHUNK_ROUNDS = 4        # <=512 rows per dma_gather

_cache = {}


def _prep(src, dst):
    E = src.shape[0]
    outdeg = np.bincount(src, minlength=N)
    order = np.argsort(-outdeg, kind="stable")
    real_pid = (np.arange(NC)[:, None] * CAP + np.arange(REAL)[None, :]).ravel()
    n_lo_real = int((real_pid < LO).sum())
    lo_nodes = order[:n_lo_real]
    hi_nodes = order[n_lo_real:]
    is_lo = np.zeros(N, bool)
    is_lo[lo_nodes] = True

    l_cnt = np.bincount(dst[is_lo[src]], minlength=N)
    h_cnt = np.bincount(dst[~is_lo[src]], minlength=N)
    key = l_cnt * 1024 + h_cnt

    # lo capacity per core (real slots with pid < LO)
    lo_cap = [int(((c * CAP + np.arange(REAL)) < LO).sum()) for c in range(NC)]
    node_pid = np.full(N, -1, np.int64)
    fill = [0] * NC  # next local real slot per core

    def deal(nodes, cores, caps):
        left = {c: caps[i] for i, c in enumerate(cores)}
        active = [c for c in cores if left[c] > 0]
        i = 0
        for nd in nodes:
            c = active[i % len(active)]
            node_pid[nd] = c * CAP + fill[c]
            fill[c] += 1
            left[c] -= 1
            if left[c] == 0:
                active = [cc for cc in active if cc != c]
            i += 1

    lo_sorted = lo_nodes[np.argsort(key[lo_nodes], kind="stable")]
    hi_sorted = hi_nodes[np.argsort(key[hi_nodes], kind="stable")]
    deal(lo_sorted, list(range(6)), lo_cap[:6])
    deal(hi_sorted, [5, 6, 7], [REAL - lo_cap[5], REAL, REAL])
    assert (node_pid[np.arange(N)] >= 0).all()

    # per (core, rank): R_lo/R_hi = max l/h over the 128 nodes, max over cores
    lmax = np.zeros((NC, NBLK), np.int64)
    hmax = np.zeros((NC, NBLK), np.int64)
    loc = node_pid % CAP
    cor = node_pid // CAP
    for nd in range(0):
        pass
    np.maximum.at(lmax, (cor, loc // P), l_cnt)
    np.maximum.at(hmax, (cor, loc // P), h_cnt)
    R_lo = lmax.max(axis=0)
    R_hi = hmax.max(axis=0)
    for k in range(NBLK):
        if R_lo[k] + R_hi[k] == 0:
            R_hi[k] = 1
    RL_OFF = np.r_[0, np.cumsum(R_lo)][:-1]
    RH_OFF = np.r_[0, np.cumsum(R_hi)][:-1]
    SUM_RL, SUM_RH = int(R_lo.sum()), int(R_hi.sum())

    pid_src = node_pid[src]
    pid_dst = node_pid[dst]
    lo_e = pid_src < LO
    okey = pid_dst * 2 + (~lo_e)
    eord = np.argsort(okey, kind="stable")
    ps_s, pd_s, lo_s = pid_src[eord], pid_dst[eord], lo_e[eord]
    grp = okey[eord]
    starts = np.r_[0, np.flatnonzero(np.diff(grp)) + 1]
    seg_len = np.diff(np.r_[starts, E])
    within = np.arange(E) - np.repeat(starts, seg_len)

    c_d = pd_s // CAP
    loc_d = pd_s % CAP
    k_d = loc_d // P
    p_d = loc_d % P

    lo_flat = np.full((NC, SUM_RL, P), LO_DUMMY, np.int32)
    hi_flat = np.full((NC, SUM_RH, P), HI_DUMMY - LO, np.int32)
    m = lo_s
    lo_flat[c_d[m], RL_OFF[k_d[m]] + within[m], p_d[m]] = ps_s[m]
    m = ~lo_s
    hi_flat[c_d[m], RH_OFF[k_d[m]] + within[m], p_d[m]] = ps_s[m] - LO

    # chunk + wrap into dma_gather idx layout
    chunks = []   # (rank, t0_in_block, nt, is_lo, w_off)
    w_off = 0
    parts = [[] for _ in range(NC)]
    for k in range(NBLK):
        for is_l, R, OFF, flat in ((True, R_lo, RL_OFF, lo_flat), (False, R_hi, RH_OFF, hi_flat)):
            t0 = 0
            while t0 < R[k]:
                nt = min(CHUNK_ROUNDS, R[k] - t0)
                for c in range(NC):
                    arr = flat[c, OFF[k] + t0: OFF[k] + t0 + nt, :].reshape(nt * P)
                    w = arr.reshape(nt * 8, 16).T.astype(np.int16)   # [16, nt*8]
                    parts[c].append(np.tile(w, (8, 1)))              # [128, nt*8]
                chunks.append((k, (0 if is_l else int(R_lo[k])) + t0, nt, is_l, w_off))
                w_off += nt * 8
                t0 += nt
    idxw = np.stack([np.concatenate(parts[c], axis=1) for c in range(NC)])  # [NC,128,NW]

    # inverse map for output assembly: original node for each (core, local<REAL)
    node_of = np.zeros(NTOT, np.int64)
    node_of[node_pid] = np.arange(N)
    return dict(node_pid=node_pid, node_of=node_of, R_lo=R_lo, R_hi=R_hi,
                chunks=chunks, idxw=idxw, NW=w_off)


def _build(R_lo, R_hi, NW):
    import concourse.bacc as bacc
    import concourse.tile as tile
    import concourse.bass as bass
    from concourse import mybir
    from concourse.library_config import mlp
    f16, f32, i16 = mybir.dt.float16, mybir.dt.float32, mybir.dt.int16
    AF = mybir.ActivationFunctionType
    OP = mybir.AluOpType

    nc = bacc.Bacc("TRN2", target_bir_lowering=False,
                   dynamic_dma_scratch_size=65536, num_swdge_queues=4)
    xT = nc.dram_tensor("xT", [P, CAP], f16, kind="ExternalInput")
    idxw = nc.dram_tensor("idxw", [P, NW], i16, kind="ExternalInput")
    W1e = nc.dram_tensor("W1e", [P, 272], f16, kind="ExternalInput")
    W2ea = nc.dram_tensor("W2ea", [P, 272], f16, kind="ExternalInput")
    W2eb = nc.dram_tensor("W2eb", [P, 272], f16, kind="ExternalInput")
    b1r = nc.dram_tensor("b1r", [P, 256], f32, kind="ExternalInput")
    b2r = nc.dram_tensor("b2r", [P, 256], f32, kind="ExternalInput")
    idn = nc.dram_tensor("idn", [P, P], f16, kind="ExternalInput")
    elneg = nc.dram_tensor("elneg", [1, 16], f16, kind="ExternalInput")
    out2 = nc.dram_tensor("out2", [CAP, 256], f32, kind="ExternalOutput")

    t1s = nc.dram_tensor("t1s", [CAP, ROWE], f16)
    er1s = nc.dram_tensor("er1s", [CAP, 8], f32)
    t2s = nc.dram_tensor("t2s", [CAP, ROWE], f16)
    er2s = nc.dram_tensor("er2s", [CAP, 8], f32)
    t1f = nc.dram_tensor("t1f", [NTOT, ROWE], f16, addr_space="Shared")
    er1f = nc.dram_tensor("er1f", [NTOT, 8], f32, addr_space="Shared")
    t2f = nc.dram_tensor("t2f", [NTOT, ROWE], f16, addr_space="Shared")
    er2f = nc.dram_tensor("er2f", [NTOT, 8], f32, addr_space="Shared")

    rg = [list(range(NC))]
    chunks = _cache["prep"]["chunks"]
    qn = [0]

    with tile.TileContext(nc) as tc:
        nc.gpsimd.load_library(mlp)
        with tc.tile_pool(name="con", bufs=1) as con, \
             tc.tile_pool(name="wp", bufs=2) as wp, \
             tc.tile_pool(name="ixp", bufs=4) as ixp, \
             tc.tile_pool(name="gp", bufs=2) as gp, \
             tc.tile_pool(name="ap", bufs=2) as apool, \
             tc.tile_pool(name="Gp", bufs=3) as Gp, \
             tc.tile_pool(name="psA", bufs=2, space="PSUM") as psA, \
             tc.tile_pool(name="psB", bufs=2, space="PSUM") as psB:

            ident = con.tile([P, P], f16)
            nc.sync.dma_start(out=ident[:], in_=idn[:])
            w1 = con.tile([P, 272], f16)
            nc.sync.dma_start(out=w1[:], in_=W1e[:])
            w2a = con.tile([P, 272], f16)
            nc.sync.dma_start(out=w2a[:], in_=W2ea[:])
            w2b = con.tile([P, 272], f16)
            nc.sync.dma_start(out=w2b[:], in_=W2eb[:])
            b1 = con.tile([P, 256], f32)
            nc.sync.dma_start(out=b1[:], in_=b1r[:])
            b2 = con.tile([P, 256], f32)
            nc.sync.dma_start(out=b2[:], in_=b2r[:])
            eneg = con.tile([1, 16], f16)
            nc.sync.dma_start(out=eneg[:], in_=elneg[:])

            def emit_row(ps, ts, ers, k):
                """ps [128,272] psum -> write table row + er shard for block k."""
                row = wp.tile([P, ROWE], f16, tag="row")
                nc.vector.tensor_copy(out=row[:, 0:256], in_=ps[:, 0:256])
                rf = row[:].bitcast(f32)
                nc.vector.tensor_copy(out=rf[:, 128:136], in_=ps[:, 256:264])
                erb = wp.tile([P, 8], f32, tag="erw")
                nc.vector.tensor_copy(out=erb[:], in_=ps[:, 264:272])
                nc.sync.dma_start(out=ts[k*P:(k+1)*P, :], in_=row[:])
                nc.sync.dma_start(out=ers[k*P:(k+1)*P, :], in_=erb[:])

            # ---- table 1 build ----
            for k in range(NBLK):
                xt = wp.tile([P, P], f16, tag="xt")
                nc.sync.dma_start(out=xt[:], in_=xT[:, k*P:(k+1)*P])
                ps = psB.tile([P, 272], f32, tag="tb")
                nc.tensor.matmul(out=ps[:], lhsT=xt[:], rhs=w1[:], start=True, stop=True)
                emit_row(ps, t1s, er1s, k)

            nc.sync.dma_start(out=t1s[6250:6251, 256:272], in_=eneg[:])
            nc.gpsimd.collective_compute(
                "AllGather", mybir.AluOpType.bypass, replica_groups=rg,
                ins=[t1s[:]], outs=[t1f[:]])
            nc.gpsimd.collective_compute(
                "AllGather", mybir.AluOpType.bypass, replica_groups=rg,
                ins=[er1s[:]], outs=[er1f[:]])

            def edge_layer(tf, ers_shard, layer):
                for k in range(NBLK):
                    Rl, Rh = int(R_lo[k]), int(R_hi[k])
                    T = Rl + Rh
                    gbuf = gp.tile([P, T, ROWE], f16, tag="g")
                    for (ck, t0, nt, is_l, woff) in chunks:
                        if ck != k:
                            continue
                        it = ixp.tile([P, nt * 8], i16, tag="ix")
                        nc.sync.dma_start(out=it[:], in_=idxw[:, woff:woff + nt*8])
                        srcap = tf[0:LO, :] if is_l else tf[LO:NTOT, :]
                        nc.gpsimd.dma_gather(gbuf[:, t0:t0+nt, :], srcap, it[:],
                                             nt * P, nt * P, ROWE,
                                             queue_num=qn[0] % 4)
                        qn[0] += 1
                    erb = apool.tile([P, 8], f32, tag="erb")
                    nc.sync.dma_start(out=erb[:], in_=ers_shard[k*P:(k+1)*P, :])

                    elv = gbuf[:].bitcast(f32)[:, :, 128:136]       # [128,T,8]
                    e = apool.tile([P, T * 8], f32, tag="e")
                    e3 = e[:].rearrange("p (t h) -> p t h", h=8)
                    nc.vector.tensor_tensor(
                        out=e3, in0=elv,
                        in1=erb[:, None, :].to_broadcast([P, T, 8]), op=OP.add)
                    esc = apool.tile([P, T * 8], f32, tag="esc")
                    nc.vector.tensor_scalar_mul(out=esc[:], in0=e[:], scalar1=0.2)
                    nc.vector.tensor_tensor(out=e[:], in0=e[:], in1=esc[:], op=OP.max)
                    mx = apool.tile([P, 8], f32, tag="mx")
                    eht = e[:].rearrange("p (t h) -> p h t", h=8)
                    nc.vector.reduce_max(out=mx[:], in_=eht, axis=mybir.AxisListType.X)
                    nc.vector.tensor_tensor(
                        out=e3, in0=e3,
                        in1=mx[:, None, :].to_broadcast([P, T, 8]), op=OP.subtract)
                    ex = apool.tile([P, T * 8], f16, tag="ex")
                    nc.scalar.activation(out=ex[:], in_=e[:], func=AF.Exp)
                    s = apool.tile([P, 8], f32, tag="s")
                    nc.vector.reduce_sum(
                        out=s[:], in_=ex[:].rearrange("p (t h) -> p h t", h=8),
                        axis=mybir.AxisListType.X)
                    nc.vector.tensor_scalar_max(out=s[:], in0=s[:], scalar1=1e-30)
                    rs = apool.tile([P, 8], f32, tag="rs")
                    nc.vector.reciprocal(out=rs[:], in_=s[:])

                    pagg = psA.tile([P, 256], f32, tag="agg")
                    for t in range(T):
                        G = Gp.tile([P, 256], f16, tag="G")
                        g3 = G[:].rearrange("p (h d) -> p h d", h=8)
                        exv = ex[:].rearrange("p (t h) -> p t h", h=8)[:, t, :, None]
                        nc.vector.tensor_tensor(
                            out=g3,
                            in0=gbuf[:, t, 0:256].rearrange("p (h d) -> p h d", h=8),
                            in1=exv.to_broadcast([P, 8, D]), op=OP.mult)
                        nc.tensor.matmul(out=pagg[:], lhsT=ident[:], rhs=G[:],
                                         start=(t == 0), stop=(t == T - 1))

                    o1 = wp.tile([P, 256], f32, tag="o1")
                    nc.vector.tensor_tensor(
                        out=o1[:].rearrange("p (h d) -> p h d", h=8),
                        in0=pagg[:].rearrange("p (h d) -> p h d", h=8),
                        in1=rs[:, :, None].to_broadcast([P, 8, D]), op=OP.mult)
                    bias = b1 if layer == 1 else b2
                    nc.vector.tensor_tensor(out=o1[:], in0=o1[:], in1=bias[:], op=OP.add)
                    if layer == 2:
                        nc.sync.dma_start(out=out2[k*P:(k+1)*P, :], in_=o1[:])
                        continue
                    # ELU -> h2 fp16
                    ea = wp.tile([P, 256], f32, tag="ea")
                    nc.scalar.activation(out=ea[:], in_=o1[:], func=AF.Exp)
                    r2 = wp.tile([P, 256], f32, tag="r2")
                    nc.scalar.activation(out=r2[:], in_=ea[:], func=AF.Relu,
                                         scale=-1.0, bias=1.0)
                    rx = wp.tile([P, 256], f32, tag="rx")
                    nc.scalar.activation(out=rx[:], in_=o1[:], func=AF.Relu)
                    h2 = wp.tile([P, 256], f16, tag="h2")
                    nc.vector.tensor_tensor(out=h2[:], in0=rx[:], in1=r2[:], op=OP.subtract)
                    ps2 = psB.tile([P, 272], f32, tag="tb")
                    for cc in range(2):
                        pst = psA.tile([P, P], f16, tag="tr")
                        nc.tensor.transpose(out=pst[:], in_=h2[:, cc*P:(cc+1)*P],
                                            identity=ident[:])
                        hT = wp.tile([P, P], f16, tag="hT")
                        nc.vector.tensor_copy(out=hT[:], in_=pst[:])
                        nc.tensor.matmul(out=ps2[:], lhsT=hT[:],
                                         rhs=(w2a if cc == 0 else w2b)[:],
                                         start=(cc == 0), stop=(cc == 1))
                    emit_row(ps2, t2s, er2s, k)

            edge_layer(t1f, er1s, 1)
            nc.sync.dma_start(out=t2s[6250:6251, 256:272], in_=eneg[:])
            nc.gpsimd.collective_compute(
                "AllGather", mybir.AluOpType.bypass, replica_groups=rg,
                ins=[t2s[:]], outs=[t2f[:]])
            nc.gpsimd.collective_compute(
                "AllGather", mybir.AluOpType.bypass, replica_groups=rg,
                ins=[er2s[:]], outs=[er2f[:]])
            edge_layer(t2f, er2s, 2)

    nc.compile()
    return nc


def kernel(in_feat, src, dst, W1, a_l1, a_r1, b1, W2, a_l2, a_r2, b2):
    from concourse.bass_utils import run_bass_kernel_spmd
    in_feat = np.asarray(in_feat, np.float32)
    src = np.asarray(src, np.int32)
    dst = np.asarray(dst, np.int32)
    key = "prep"
    if key not in _cache:
        _cache[key] = _prep(src, dst)
        _cache["nc"] = _build(_cache[key]["R_lo"], _cache[key]["R_hi"],
                              _cache[key]["NW"])
    pp = _cache[key]
    nc = _cache["nc"]

    node_pid = pp["node_pid"]
    x_p = np.zeros((NTOT, IN_F), np.float32)
    x_p[node_pid] = in_feat
    W1 = np.asarray(W1, np.float32)
    W2 = np.asarray(W2, np.float32)
    wa_l1 = np.einsum("fhd,hd->fh", W1.reshape(IN_F, H, D), np.asarray(a_l1, np.float32))
    wa_r1 = np.einsum("fhd,hd->fh", W1.reshape(IN_F, H, D), np.asarray(a_r1, np.float32))
    wa_l2 = np.einsum("fhd,hd->fh", W2.reshape(HID, H, D), np.asarray(a_l2, np.float32))
    wa_r2 = np.einsum("fhd,hd->fh", W2.reshape(HID, H, D), np.asarray(a_r2, np.float32))
    W1e = np.concatenate([W1, wa_l1, wa_r1], axis=1).astype(np.float16)
    W2e = np.concatenate([W2, wa_l2, wa_r2], axis=1).astype(np.float16)
    b1rep = np.tile(np.asarray(b1, np.float32)[None, :], (P, 1))
    b2rep = np.tile(np.asarray(b2, np.float32)[None, :], (P, 1))
    idn = np.eye(P, dtype=np.float16)

    in_maps = []
    for c in range(NC):
        xT_c = np.ascontiguousarray(x_p[c*CAP:(c+1)*CAP].T.astype(np.float16))
        in_maps.append({
            "xT": xT_c, "idxw": pp["idxw"][c],
            "W1e": W1e, "W2ea": W2e[:P], "W2eb": W2e[P:],
            "b1r": b1rep, "b2r": b2rep, "idn": idn,
            "elneg": np.full(8, -1e30, np.float32).view(np.float16).reshape(1, 16),
        })
    res = run_bass_kernel_spmd(nc, in_maps, list(range(NC)))
    out = np.zeros((N, HID), np.float32)
    node_of = pp["node_of"]
    for c in range(NC):
        o = res.results[c]["out2"]
        ids = node_of[c*CAP + np.arange(REAL)]
        out[ids] = o[:REAL]
    return out


# revision 12
# speedup vs baseline: 82.1864x; 82.1864x over previous
"""2-layer GAT (8 heads x 32) on 8 Trainium2 NeuronCores.

Strategy (dst-major "round" tiling):
 - Host permutes nodes into 8 core-shards x 49 blocks x 128 slots, packing
   nodes with similar (lo,hi) in-degree into the same block rank so the
   per-rank round counts (ELL padding) are small and identical across cores
   (single SPMD NEFF).
 - Gather tables hold per-node rows [h_proj fp16 (256) | el fp32 (8) | pad]
   = 768B, gathered per edge with the TIE-accelerated dma_gather (int16
   indices -> table split at row 32768 into lo/hi halves; each node's
   lo-src edges occupy the first R_lo rounds of its block, hi-src edges the
   rest, padded with dummy rows whose el = -1e30 so exp() -> 0).
 - Round tile t of a block holds edge t of each of the 128 dst nodes =>
   attention is fully dst-major: er is a direct per-partition [128,8] tile,
   softmax max/sum are strided free-dim reductions, and the weighted
   aggregation is an identity-lhsT matmul accumulating G_t into PSUM.
 - Tables are built shard-local (x^T @ [W|wa_l|wa_r] fused matmul) and
   AllGather'd between layers.
"""
import sys
sys.path.insert(0, "/opt/trn_rl_repo")
import numpy as np

N = 50000
E_FULL = 850000
IN_F = 128
H = 8
D = 32
HID = 256
P = 128
NC = 8
NBLK = 49
CAP = NBLK * P          # 6272 slots per core
REAL = 6250             # real nodes per core
NTOT = NC * CAP         # 50176
LO = 32768
ROWE = 384              # fp16 elems per table row (768B)
LO_DUMMY = 6250         # core0 spare slot (block 48, slot 106)
HI_DUMMY = 7 * CAP + 6250   # 50154 -> hi-relative 17386
CHUNK_ROUNDS = 4        # <=512 rows per dma_gather

_cache = {}


def _prep(src, dst):
    E = src.shape[0]
    outdeg = np.bincount(src, minlength=N)
    order = np.argsort(-outdeg, kind="stable")
    real_pid = (np.arange(NC)[:, None] * CAP + np.arange(REAL)[None, :]).ravel()
    n_lo_real = int((real_pid < LO).sum())
    lo_nodes = order[:n_lo_real]
    hi_nodes = order[n_lo_real:]
    is_lo = np.zeros(N, bool)
    is_lo[lo_nodes] = True

    l_cnt = np.bincount(dst[is_lo[src]], minlength=N)
    h_cnt = np.bincount(dst[~is_lo[src]], minlength=N)
    key = l_cnt * 1024 + h_cnt

    # lo capacity per core (real slots with pid < LO)
    lo_cap = [int(((c * CAP + np.arange(REAL)) < LO).sum()) for c in range(NC)]
    node_pid = np.full(N, -1, np.int64)
    fill = [0] * NC  # next local real slot per core

    def deal(nodes, cores, caps):
        left = {c: caps[i] for i, c in enumerate(cores)}
        active = [c for c in cores if left[c] > 0]
        i = 0
        for nd in nodes:
            c = active[i % len(active)]
            node_pid[nd] = c * CAP + fill[c]
            fill[c] += 1
            left[c] -= 1
            if left[c] == 0:
                active = [cc for cc in active if cc != c]
            i += 1

    lo_sorted = lo_nodes[np.argsort(key[lo_nodes], kind="stable")]
    hi_sorted = hi_nodes[np.argsort(key[hi_nodes], kind="stable")]
    deal(lo_sorted, list(range(6)), lo_cap[:6])
    deal(hi_sorted, [5, 6, 7], [REAL - lo_cap[5], REAL, REAL])
    assert (node_pid[np.arange(N)] >= 0).all()

    # per (core, rank): R_lo/R_hi = max l/h over the 128 nodes, max over cores
    lmax = np.zeros((NC, NBLK), np.int64)
    hmax = np.zeros((NC, NBLK), np.int64)
    loc = node_pid % CAP
    cor = node_pid // CAP
    for nd in range(0):
        pass
    np.maximum.at(lmax, (cor, loc // P), l_cnt)
    np.maximum.at(hmax, (cor, loc // P), h_cnt)
    R_lo = lmax.max(axis=0)
    R_hi = hmax.max(axis=0)
    for k in range(NBLK):
        if R_lo[k] + R_hi[k] == 0:
            R_hi[k] = 1
    RL_OFF = np.r_[0, np.cumsum(R_lo)][:-1]
    RH_OFF = np.r_[0, np.cumsum(R_hi)][:-1]
    SUM_RL, SUM_RH = int(R_lo.sum()), int(R_hi.sum())

    pid_src = node_pid[src]
    pid_dst = node_pid[dst]
    lo_e = pid_src < LO
    okey = pid_dst * 2 + (~lo_e)
    eord = np.argsort(okey, kind="stable")
    ps_s, pd_s, lo_s = pid_src[eord], pid_dst[eord], lo_e[eord]
    grp = okey[eord]
    starts = np.r_[0, np.flatnonzero(np.diff(grp)) + 1]
    seg_len = np.diff(np.r_[starts, E])
    within = np.arange(E) - np.repeat(starts, seg_len)

    c_d = pd_s // CAP
    loc_d = pd_s % CAP
    k_d = loc_d // P
    p_d = loc_d % P

    lo_flat = np.full((NC, SUM_RL, P), LO_DUMMY, np.int32)
    hi_flat = np.full((NC, SUM_RH, P), HI_DUMMY - LO, np.int32)
    m = lo_s
    lo_flat[c_d[m], RL_OFF[k_d[m]] + within[m], p_d[m]] = ps_s[m]
    m = ~lo_s
    hi_flat[c_d[m], RH_OFF[k_d[m]] + within[m], p_d[m]] = ps_s[m] - LO

    # chunk + wrap into dma_gather idx layout
    chunks = []   # (rank, t0_in_block, nt, is_lo, w_off)
    w_off = 0
    parts = [[] for _ in range(NC)]
    for k in range(NBLK):
        for is_l, R, OFF, flat in ((True, R_lo, RL_OFF, lo_flat), (False, R_hi, RH_OFF, hi_flat)):
            t0 = 0
            while t0 < R[k]:
                nt = min(CHUNK_ROUNDS, R[k] - t0)
                for c in range(NC):
                    arr = flat[c, OFF[k] + t0: OFF[k] + t0 + nt, :].reshape(nt * P)
                    w = arr.reshape(nt * 8, 16).T.astype(np.int16)   # [16, nt*8]
                    parts[c].append(np.tile(w, (8, 1)))              # [128, nt*8]
                chunks.append((k, (0 if is_l else int(R_lo[k])) + t0, nt, is_l, w_off))
                w_off += nt * 8
                t0 += nt
    idxw = np.stack([np.concatenate(parts[c], axis=1) for c in range(NC)])  # [NC,128,NW]

    # inverse map for output assembly: original node for each (core, local<REAL)
    node_of = np.zeros(NTOT, np.int64)
    node_of[node_pid] = np.arange(N)
    return dict(node_pid=node_pid, node_of=node_of, R_lo=R_lo, R_hi=R_hi,
                chunks=chunks, idxw=idxw, NW=w_off)


def _build(R_lo, R_hi, NW):
    import concourse.bacc as bacc
    import concourse.tile as tile
    import concourse.bass as bass
    from concourse import mybir
    from concourse.library_config import mlp
    f16, f32, i16 = mybir.dt.float16, mybir.dt.float32, mybir.dt.int16
    AF = mybir.ActivationFunctionType
    OP = mybir.AluOpType

    nc = bacc.Bacc("TRN2", target_bir_lowering=False,
                   dynamic_dma_scratch_size=65536, num_swdge_queues=4)
    xT = nc.dram_tensor("xT", [P, CAP], f16, kind="ExternalInput")
    idxw = nc.dram_tensor("idxw", [P, NW], i16, kind="ExternalInput")
    W1e = nc.dram_tensor("W1e", [P, 272], f16, kind="ExternalInput")
    W2ea = nc.dram_tensor("W2ea", [P, 272], f16, kind="ExternalInput")
    W2eb = nc.dram_tensor("W2eb", [P, 272], f16, kind="ExternalInput")
    b1r = nc.dram_tensor("b1r", [P, 256], f32, kind="ExternalInput")
    b2r = nc.dram_tensor("b2r", [P, 256], f32, kind="ExternalInput")
    idn = nc.dram_tensor("idn", [P, P], f16, kind="ExternalInput")
    elneg = nc.dram_tensor("elneg", [1, 16], f16, kind="ExternalInput")
    out2 = nc.dram_tensor("out2", [CAP, 256], f32, kind="ExternalOutput")

    t1s = nc.dram_tensor("t1s", [CAP, ROWE], f16)
    er1s = nc.dram_tensor("er1s", [CAP, 8], f32)
    t2s = nc.dram_tensor("t2s", [CAP, ROWE], f16)
    er2s = nc.dram_tensor("er2s", [CAP, 8], f32)
    t1f = nc.dram_tensor("t1f", [NTOT, ROWE], f16, addr_space="Shared")
    er1f = nc.dram_tensor("er1f", [NTOT, 8], f32, addr_space="Shared")
    t2f = nc.dram_tensor("t2f", [NTOT, ROWE], f16, addr_space="Shared")
    er2f = nc.dram_tensor("er2f", [NTOT, 8], f32, addr_space="Shared")

    rg = [list(range(NC))]
    chunks = _cache["prep"]["chunks"]
    qn = [0]

    with tile.TileContext(nc) as tc:
        nc.gpsimd.load_library(mlp)
        with tc.tile_pool(name="con", bufs=1) as con, \
             tc.tile_pool(name="wp", bufs=2) as wp, \
             tc.tile_pool(name="ixp", bufs=4) as ixp, \
             tc.tile_pool(name="gp", bufs=2) as gp, \
             tc.tile_pool(name="ap", bufs=2) as apool, \
             tc.tile_pool(name="Gp", bufs=3) as Gp, \
             tc.tile_pool(name="psA", bufs=2, space="PSUM") as psA, \
             tc.tile_pool(name="psB", bufs=2, space="PSUM") as psB:

            ident = con.tile([P, P], f16)
            nc.sync.dma_start(out=ident[:], in_=idn[:])
            w1 = con.tile([P, 272], f16)
            nc.sync.dma_start(out=w1[:], in_=W1e[:])
            w2a = con.tile([P, 272], f16)
            nc.sync.dma_start(out=w2a[:], in_=W2ea[:])
            w2b = con.tile([P, 272], f16)
            nc.sync.dma_start(out=w2b[:], in_=W2eb[:])
            b1 = con.tile([P, 256], f32)
            nc.sync.dma_start(out=b1[:], in_=b1r[:])
            b2 = con.tile([P, 256], f32)
            nc.sync.dma_start(out=b2[:], in_=b2r[:])
            eneg = con.tile([1, 16], f16)
            nc.sync.dma_start(out=eneg[:], in_=elneg[:])

            def emit_row(ps, ts, ers, k):
                """ps [128,272] psum -> write table row + er shard for block k."""
                row = wp.tile([P, ROWE], f16, tag="row")
                nc.vector.tensor_copy(out=row[:, 0:256], in_=ps[:, 0:256])
                rf = row[:].bitcast(f32)
                nc.vector.tensor_copy(out=rf[:, 128:136], in_=ps[:, 256:264])
                erb = wp.tile([P, 8], f32, tag="erw")
                nc.vector.tensor_copy(out=erb[:], in_=ps[:, 264:272])
                nc.sync.dma_start(out=ts[k*P:(k+1)*P, :], in_=row[:])
                nc.sync.dma_start(out=ers[k*P:(k+1)*P, :], in_=erb[:])

            # ---- table 1 build ----
            for k in range(NBLK):
                xt = wp.tile([P, P], f16, tag="xt")
                nc.sync.dma_start(out=xt[:], in_=xT[:, k*P:(k+1)*P])
                ps = psB.tile([P, 272], f32, tag="tb")
                nc.tensor.matmul(out=ps[:], lhsT=xt[:], rhs=w1[:], start=True, stop=True)
                emit_row(ps, t1s, er1s, k)

            nc.sync.dma_start(out=t1s[6250:6251, 256:272], in_=eneg[:])
            nc.gpsimd.collective_compute(
                "AllGather", mybir.AluOpType.bypass, replica_groups=rg,
                ins=[t1s[:]], outs=[t1f[:]])
            nc.gpsimd.collective_compute(
                "AllGather", mybir.AluOpType.bypass, replica_groups=rg,
                ins=[er1s[:]], outs=[er1f[:]])

            def edge_layer(tf, ers_shard, layer):
                for k in range(NBLK):
                    Rl, Rh = int(R_lo[k]), int(R_hi[k])
                    T = Rl + Rh
                    gbuf = gp.tile([P, T, ROWE], f16, tag="g")
                    for (ck, t0, nt, is_l, woff) in chunks:
                        if ck != k:
                            continue
                        it = ixp.tile([P, nt * 8], i16, tag="ix")
                        nc.sync.dma_start(out=it[:], in_=idxw[:, woff:woff + nt*8])
                        srcap = tf[0:LO, :] if is_l else tf[LO:NTOT, :]
                        nc.gpsimd.dma_gather(gbuf[:, t0:t0+nt, :], srcap, it[:],
                                             nt * P, nt * P, ROWE,
                                             queue_num=qn[0] % 4)
                        qn[0] += 1
                    erb = apool.tile([P, 8], f32, tag="erb")
                    nc.sync.dma_start(out=erb[:], in_=ers_shard[k*P:(k+1)*P, :])

                    elv = gbuf[:].bitcast(f32)[:, :, 128:136]       # [128,T,8]
                    e = apool.tile([P, T * 8], f32, tag="e")
                    e3 = e[:].rearrange("p (t h) -> p t h", h=8)
                    nc.vector.tensor_tensor(
                        out=e3, in0=elv,
                        in1=erb[:, None, :].to_broadcast([P, T, 8]), op=OP.add)
                    esc = apool.tile([P, T * 8], f32, tag="esc")
                    nc.vector.tensor_scalar_mul(out=esc[:], in0=e[:], scalar1=0.2)
                    nc.vector.tensor_tensor(out=e[:], in0=e[:], in1=esc[:], op=OP.max)
                    mx = apool.tile([P, 8], f32, tag="mx")
                    eht = e[:].rearrange("p (t h) -> p h t", h=8)
                    nc.vector.reduce_max(out=mx[:], in_=eht, axis=mybir.AxisListType.X)
                    nc.vector.tensor_tensor(
                        out=e3, in0=e3,
                        in1=mx[:, None, :].to_broadcast([P, T, 8]), op=OP.subtract)
                    ex = apool.tile([P, T * 8], f16, tag="ex")
                    nc.scalar.activation(out=ex[:], in_=e[:], func=AF.Exp)
                    s = apool.tile([P, 8], f32, tag="s")
                    nc.vector.reduce_sum(
                        out=s[:], in_=ex[:].rearrange("p (t h) -> p h t", h=8),
                        axis=mybir.AxisListType.X)
                    nc.vector.tensor_scalar_max(out=s[:], in0=s[:], scalar1=1e-30)
                    rs = apool.tile([P, 8], f32, tag="rs")
                    nc.vector.reciprocal(out=rs[:], in_=s[:])

                    pagg = psA.tile([P, 256], f32, tag="agg")
                    for t in range(T):
                        G = Gp.tile([P, 256], f16, tag="G")
                        g3 = G[:].rearrange("p (h d) -> p h d", h=8)
                        exv = ex[:].rearrange("p (t h) -> p t h", h=8)[:, t, :, None]
                        nc.vector.tensor_tensor(
                            out=g3,
                            in0=gbuf[:, t, 0:256].rearrange("p (h d) -> p h d", h=8),
                            in1=exv.to_broadcast([P, 8, D]), op=OP.mult)
                        nc.tensor.matmul(out=pagg[:], lhsT=ident[:], rhs=G[:],
                                         start=(t == 0), stop=(t == T - 1))

                    o1 = wp.tile([P, 256], f32, tag="o1")
                    nc.vector.tensor_tensor(
                        out=o1[:].rearrange("p (h d) -> p h d", h=8),
                        in0=pagg[:].rearrange("p (h d) -> p h d", h=8),
                        in1=rs[:, :, None].to_broadcast([P, 8, D]), op=OP.mult)
                    bias = b1 if layer == 1 else b2
                    nc.vector.tensor_tensor(out=o1[:], in0=o1[:], in1=bias[:], op=OP.add)
                    if layer == 2:
                        nc.sync.dma_start(out=out2[k*P:(k+1)*P, :], in_=o1[:])
                        continue
                    # ELU -> h2 fp16
                    ea = wp.tile([P, 256], f32, tag="ea")
                    nc.scalar.activation(out=ea[:], in_=o1[:], func=AF.Exp)
                    r2 = wp.tile([P, 256], f32, tag="r2")
                    nc.scalar.activation(out=r2[:], in_=ea[:], func=AF.Relu,
                                         scale=-1.0, bias=1.0)
                    rx = wp.tile([P, 256], f32, tag="rx")
                    nc.scalar.activation(out=rx[:], in_=o1[:], func=AF.Relu)
                    h2 = wp.tile([P, 256], f16, tag="h2")
                    nc.vector.tensor_tensor(out=h2[:], in0=rx[:], in1=r2[:], op=OP.subtract)
                    ps2 = psB.tile([P, 272], f32, tag="tb")
                    for cc in range(2):
                        pst = psA.tile([P, P], f16, tag="tr")
                        nc.tensor.transpose(out=pst[:], in_=h2[:, cc*P:(cc+1)*P],
                                            identity=ident[:])
                        hT = wp.tile([P, P], f16, tag="hT")
                        nc.vector.tensor_copy(out=hT[:], in_=pst[:])
                        nc.tensor.matmul(out=ps2[:], lhsT=hT[:],
                                         rhs=(w2a if cc == 0 else w2b)[:],
                                         start=(cc == 0), stop=(cc == 1))
                    emit_row(ps2, t2s, er2s, k)

            edge_layer(t1f, er1s, 1)
            nc.sync.dma_start(out=t2s[6250:6251, 256:272], in_=eneg[:])
            nc.gpsimd.collective_compute(
                "AllGather", mybir.AluOpType.bypass, replica_groups=rg,
                ins=[t2s[:]], outs=[t2f[:]])
            nc.gpsimd.collective_compute(
                "AllGather", mybir.AluOpType.bypass, replica_groups=rg,
                ins=[er2s[:]], outs=[er2f[:]])
            edge_layer(t2f, er2s, 2)

    nc.compile()
    return nc


def kernel(in_feat, src, dst, W1, a_l1, a_r1, b1, W2, a_l2, a_r2, b2):
    from concourse.bass_utils import run_bass_kernel_spmd
    in_feat = np.asarray(in_feat, np.float32)
    src = np.asarray(src, np.int32)
    dst = np.asarray(dst, np.int32)
    key = "prep"
    if key not in _cache:
        _cache[key] = _prep(src, dst)
        _cache["nc"] = _build(_cache[key]["R_lo"], _cache[key]["R_hi"],
                              _cache[key]["NW"])
    pp = _cache[key]
    nc = _cache["nc"]

    node_pid = pp["node_pid"]
    x_p = np.zeros((NTOT, IN_F), np.float32)
    x_p[node_pid] = in_feat
    W1 = np.asarray(W1, np.float32)
    W2 = np.asarray(W2, np.float32)
    wa_l1 = np.einsum("fhd,hd->fh", W1.reshape(IN_F, H, D), np.asarray(a_l1, np.float32))
    wa_r1 = np.einsum("fhd,hd->fh", W1.reshape(IN_F, H, D), np.asarray(a_r1, np.float32))
    wa_l2 = np.einsum("fhd,hd->fh", W2.reshape(HID, H, D), np.asarray(a_l2, np.float32))
    wa_r2 = np.einsum("fhd,hd->fh", W2.reshape(HID, H, D), np.asarray(a_r2, np.float32))
    W1e = np.concatenate([W1, wa_l1, wa_r1], axis=1).astype(np.float16)
    W2e = np.concatenate([W2, wa_l2, wa_r2], axis=1).astype(np.float16)
    b1rep = np.tile(np.asarray(b1, np.float32)[None, :], (P, 1))
    b2rep = np.tile(np.asarray(b2, np.float32)[None, :], (P, 1))
    idn = np.eye(P, dtype=np.float16)

    in_maps = []
    for c in range(NC):
        xT_c = np.ascontiguousarray(x_p[c*CAP:(c+1)*CAP].T.astype(np.float16))
        in_maps.append({
            "xT": xT_c, "idxw": pp["idxw"][c],
            "W1e": W1e, "W2ea": W2e[:P], "W2eb": W2e[P:],
            "b1r": b1rep, "b2r": b2rep, "idn": idn,
            "elneg": np.full(8, -1e30, np.float32).view(np.float16).reshape(1, 16),
        })
    _cache["last_in_maps"] = in_maps
    global _last_in_maps
    _last_in_maps = in_maps
    res = run_bass_kernel_spmd(nc, in_maps, list(range(NC)))
    out = np.zeros((N, HID), np.float32)
    node_of = pp["node_of"]
    for c in range(NC):
        o = res.results[c]["out2"]
        ids = node_of[c*CAP + np.arange(REAL)]
        out[ids] = o[:REAL]
    return out
